# revision 10
# baseline (speedup 1.0000x reference)
"""BitLinear FFN (BitNet b1.58) Trainium2 kernel, 8-core SPMD, token-major.

Strategy: data-parallel over tokens (1024 per core). Host ships transposed
weight/x copies (layout-only), weight quantization is sharded 1/8 per core
and shared via three per-tensor fp8 AllGathers; a tiny AllReduce combines
per-core |w| sums for the global absmean scales.

Gate/up matmuls run as fp8 DoubleRow streaks: the int8 activations are
split exactly as a = hi16 + lo (hi16 = 16*rne(a/16) in {-128..128 step 16},
lo in [-8, 8]; both exact in fp8e4), so one DoubleRow matmul contracts two
128-k-tiles at the doubled fp8 rate. Microbenched: DR streaks into a single
psum reach ~61 ns per 512-wide MM pair (vs 133 ns bf16), but interleaving
two psum accumulation groups destroys the rate - hence gate's full streak
runs before up's. The down projection stays bf16 (its operands are reused
1024x only; hi/lo split would double elementwise work for little PE gain).

The FFN middle runs in [token-partition, inner-free] layout so per-token
scales fuse into ScalarE activation scale columns. prodq 128x128 tiles are
PE-transposed back to [inner, token] as down lhsT; psum evacuations
alternate DVE/ACT to balance engines. The weight-quant epoch runs on the
otherwise-idle Pool (gpsimd) queue, and the next rep's x-quant preamble is
emitted inside the current rep's tile loop so rep boundaries stay PE-busy.

Exactness: activation ints (|.|<=127) exact in bf16/fp8-split, ternary
weights exact in fp8e4; matmuls accumulate exact integers in fp32. Rounding
uses the +/-1.5*2^23 trick (RNE).
"""

import numpy as np

import concourse.bacc as bacc
import concourse.bass as bass
import concourse.mybir as mybir
import concourse.tile as tile
from concourse.masks import make_identity

P = 128
HID = 1024
INNER = 4096
N_CORES = 8
T_CORE = 1024          # tokens per core
NT = T_CORE // P       # 8 token tiles
KI = HID // P          # 8 contraction tiles (i) for gate/up
OB = 512               # o-block width for gate/up matmuls
NOB = INNER // OB      # 8 o-blocks
KO = INNER // P        # 32 contraction tiles (o) for down
OSH = INNER // N_CORES   # 512, gate/up o-shard per core
WDSH = INNER // N_CORES  # 512, down o-row-shard per core

MROUND = 12582912.0    # 1.5 * 2**23: (v + M) - M == round-half-even(v)
M16 = 16.0 * MROUND
W_ELEMS = float(INNER * HID)
SHB = OSH * HID        # elements per staged fp8 weight shard (all 3 equal)

F32 = mybir.dt.float32
BF16 = mybir.dt.bfloat16
FP8 = mybir.dt.float8e4

A = mybir.AluOpType
AF = mybir.ActivationFunctionType
DR = mybir.MatmulPerfMode.DoubleRow

W_SRCS = None  # set inside build


def build_bass(sim_mode: bool = False, reps: int = 1):
    nc = bacc.Bacc(
        "TRN2", target_bir_lowering=False, debug=False,
        num_devices=N_CORES,
    )
    groups = [list(range(N_CORES))]

    x_d = nc.dram_tensor("x_shard", [T_CORE, HID], F32, kind="ExternalInput")
    xt_d = nc.dram_tensor("xT_shard", [HID, T_CORE], F32, kind="ExternalInput")
    wg_d = nc.dram_tensor("wgT_shard", [HID, OSH], F32, kind="ExternalInput")
    wu_d = nc.dram_tensor("wuT_shard", [HID, OSH], F32, kind="ExternalInput")
    wd_d = nc.dram_tensor("wdT_shard", [WDSH, HID], F32, kind="ExternalInput")
    out_d = nc.dram_tensor("out_shard", [T_CORE, HID], F32, kind="ExternalOutput")

    x_r = x_d.ap().rearrange("(n p) i -> n p i", p=P)        # [8, 128, 1024]
    xt_r = xt_d.ap().rearrange("(ki p) t -> ki p t", p=P)    # [8, 128, 1024]
    # big views for one-DMA weight loads: [128, sub, width]
    wg_b = wg_d.ap().rearrange("(k p) o -> p k o", p=P)      # [128, 8, 512]
    wu_b = wu_d.ap().rearrange("(k p) o -> p k o", p=P)
    wd_b = wd_d.ap().rearrange("(k p) h -> p k h", p=P)      # [128, 4, 1024]
    out_r = out_d.ap().rearrange("(n p) h -> n p h", p=P)

    w_srcs = ((wg_b, KI, OSH), (wu_b, KI, OSH), (wd_b, 4, HID))

    with tile.TileContext(nc) as tc:
        with (
            tc.tile_pool(name="const", bufs=1) as constp,
            tc.tile_pool(name="wsp", bufs=2) as wsp,       # weight-scale state
            tc.tile_pool(name="xst", bufs=2) as xstp,      # per-rep x state
            tc.tile_pool(name="xq", bufs=2) as xqp,        # xhiT/xloT fp8
            tc.tile_pool(name="wc", bufs=1) as wcp,        # fp8 weight caches
            tc.tile_pool(name="wld", bufs=2) as wldp,      # f32 weight loads
            tc.tile_pool(name="stg", bufs=1) as stgp,      # fp8 stage
            tc.tile_pool(name="stream", bufs=2) as streamp,
            tc.tile_pool(name="prod", bufs=2) as prodp,
            tc.tile_pool(name="pq", bufs=1) as pqp,
            tc.tile_pool(name="ew", bufs=2) as ewp,
            tc.tile_pool(name="tiny", bufs=2) as tinyp,
            tc.tile_pool(name="pg", bufs=4, space="PSUM") as pgp,
            tc.tile_pool(name="pd", bufs=2, space="PSUM") as pdp,
            tc.tile_pool(name="pt", bufs=2, space="PSUM") as ptp,
            tc.tile_pool(name="dram", bufs=2, space="DRAM") as dramp,
        ):
            ident = constp.tile([P, P], BF16)
            make_identity(nc, ident)
            ones_col = constp.tile([P, 1], F32)
            nc.gpsimd.memset(ones_col[:], 1.0)

            # ---------------- weight-quant epoch --------------------------
            def stage_weights():
                """|w| sums -> AllReduce -> scales -> quantize (Pool engine)
                -> three per-tensor fp8 AllGathers."""
                sums_col = wsp.tile([P, 4], F32, tag="sums")
                nc.gpsimd.memset(sums_col[:], 0.0)
                for idx, (big, nsub, w) in enumerate(w_srcs):
                    qs = nsub // 4
                    for q in range(4):
                        wld = wldp.tile([P, qs, w], F32, tag="wld")
                        nc.sync.dma_start(
                            out=wld[:], in_=big[:, q * qs:(q + 1) * qs])
                        part = tinyp.tile([P, 1], F32, tag="wabs")
                        nc.vector.tensor_reduce(
                            out=part[:], in_=wld[:].rearrange("p a b -> p (a b)"),
                            axis=mybir.AxisListType.X,
                            op=A.add, apply_absolute_value=True)
                        nc.vector.tensor_tensor(
                            out=sums_col[:, idx:idx + 1],
                            in0=sums_col[:, idx:idx + 1], in1=part[:], op=A.add)

                psums = ptp.tile([1, 4], F32, tag="pt")
                nc.tensor.matmul(psums[:], lhsT=ones_col[:], rhs=sums_col[:],
                                 start=True, stop=True)
                sums_sb = tinyp.tile([1, 4], F32)
                nc.vector.tensor_copy(out=sums_sb[:], in_=psums[:])

                # tiny AllReduce of the three |w| sums
                sums_in = dramp.tile([1, 4], F32, tag="sin")
                sums_out = dramp.tile([1, 4], F32, addr_space="Shared", tag="sout")
                nc.sync.dma_start(out=sums_in[:], in_=sums_sb[:])
                if sim_mode:
                    nc.sync.dma_start(out=sums_out[:], in_=sums_in[:])
                else:
                    nc.gpsimd.collective_compute(
                        "AllReduce", A.add, replica_groups=groups,
                        ins=[sums_in[:]], outs=[sums_out[:]])
                sums_all = tinyp.tile([1, 4], F32)
                nc.sync.dma_start(out=sums_all[:], in_=sums_out[:])

                # clip-means (= 1/s_w) and s_w, broadcast to all partitions
                mcl = tinyp.tile([1, 4], F32)
                nc.vector.tensor_scalar(
                    out=mcl[:], in0=sums_all[:], scalar1=1.0 / W_ELEMS,
                    scalar2=1e-5, op0=A.mult, op1=A.max)
                sw = tinyp.tile([1, 4], F32)
                nc.vector.reciprocal(out=sw[:], in_=mcl[:])
                swb = wsp.tile([P, 4], F32, tag="swb")
                nc.gpsimd.partition_broadcast(swb[:], sw[0:1, :])
                mclb = wsp.tile([P, 4], F32, tag="mclb")
                nc.gpsimd.partition_broadcast(mclb[:], mcl[0:1, :])
                bc_ud = wsp.tile([P, 1], F32, tag="bcud")
                nc.vector.tensor_tensor(
                    out=bc_ud[:], in0=mclb[:, 1:2], in1=mclb[:, 2:3], op=A.mult)
                nc.vector.tensor_scalar_mul(
                    bc_ud[:], bc_ud[:], 1.0 / (127.0 * 127.0))

                # quantize each tensor on the Pool queue, stage, gather
                shared_as = "Local" if sim_mode else "Shared"
                ags = []
                for idx, (big, nsub, w) in enumerate(w_srcs):
                    qs = nsub // 4
                    stage_sb = stgp.tile([P, nsub, w], FP8, tag="stg")
                    sw_col = swb[:, idx:idx + 1]
                    for q in range(4):
                        wld = wldp.tile([P, qs, w], F32, tag="wld")
                        nc.sync.dma_start(
                            out=wld[:], in_=big[:, q * qs:(q + 1) * qs])
                        fl = wld[:].rearrange("p a b -> p (a b)")
                        nc.gpsimd.tensor_scalar(
                            out=fl, in0=fl, scalar1=sw_col, scalar2=MROUND,
                            op0=A.mult, op1=A.add)
                        nc.gpsimd.tensor_scalar(
                            out=fl, in0=fl, scalar1=-MROUND, scalar2=1.0,
                            op0=A.add, op1=A.min)
                        nc.gpsimd.tensor_scalar_max(
                            stage_sb[:, q * qs:(q + 1) * qs].rearrange(
                                "p a b -> p (a b)"), fl, -1.0)
                    stg_d = dramp.tile([SHB], FP8, tag=f"stg{idx}")
                    nc.sync.dma_start(
                        out=stg_d[:].rearrange("(p s w) -> p s w", p=P, s=nsub,
                                               w=w),
                        in_=stage_sb[:])
                    ag_d = dramp.tile([N_CORES, SHB], FP8, addr_space=shared_as,
                                      tag=f"ag{idx}")
                    if sim_mode:
                        for c in range(N_CORES):
                            nc.sync.dma_start(out=ag_d[c, :], in_=stg_d[:])
                    else:
                        nc.gpsimd.collective_compute(
                            "AllGather", A.bypass, replica_groups=groups,
                            ins=[stg_d[:]], outs=[ag_d[:]])
                    ags.append(ag_d)
                return {"mclb": mclb, "bc_ud": bc_ud, "ags": ags}

            # ---------------- fp8 weight caches ---------------------------
            def emit_cache(st):
                wgT_sb = wcp.tile([P, KI, INNER], FP8, tag="wgc")
                wuT_sb = wcp.tile([P, KI, INNER], FP8, tag="wuc")
                wdT_sb = wcp.tile([P, KO, HID], FP8, tag="wdc")
                for ag, sb in ((st["ags"][0], wgT_sb), (st["ags"][1], wuT_sb)):
                    for c in range(N_CORES):
                        nc.sync.dma_start(
                            out=sb[:, :, c * OSH:(c + 1) * OSH],
                            in_=ag[c].rearrange("(p s w) -> p s w", p=P, s=KI,
                                                w=OSH))
                for c in range(N_CORES):
                    nc.sync.dma_start(
                        out=wdT_sb[:, 4 * c:4 * (c + 1), :],
                        in_=st["ags"][2][c].rearrange("(p s w) -> p s w", p=P,
                                                      s=4, w=HID))
                return {"wg": wgT_sb, "wu": wuT_sb, "wd": wdT_sb}

            # ---------------- x preamble (chunked emission) ---------------
            def xst_new():
                return {
                    "absm": xstp.tile([P, NT], F32, tag="absm", name="absm"),
                    "cg": xstp.tile([P, NT], F32, tag="cg", name="cg"),
                    "s1b": xstp.tile([P, T_CORE], F32, tag="s1b", name="s1b"),
                    "xhiT": xqp.tile([P, KI, T_CORE], FP8, tag="xhiT",
                                     name="xhiT"),
                    "xloT": xqp.tile([P, KI, T_CORE], FP8, tag="xloT",
                                     name="xloT"),
                }

            def emit_absmax_chunk(xst, ts_list):
                for ts in ts_list:
                    x_sb = streamp.tile([P, HID], F32, tag="xld")
                    nc.sync.dma_start(out=x_sb[:], in_=x_r[ts])
                    am = tinyp.tile([P, 1], F32, tag="am")
                    nc.vector.tensor_reduce(
                        out=am[:], in_=x_sb[:], axis=mybir.AxisListType.X,
                        op=A.max, apply_absolute_value=True)
                    nc.vector.tensor_scalar_max(
                        xst["absm"][:, ts:ts + 1], am[:], 1e-5)

            def emit_s1(xst, st):
                s1c = tinyp.tile([P, NT], F32, tag="s1c")
                nc.vector.reciprocal(out=s1c[:], in_=xst["absm"][:])
                nc.vector.tensor_scalar_mul(s1c[:], s1c[:], 127.0)
                s1_row = xstp.tile([1, T_CORE], F32, tag="s1row")
                for ts in range(NT):
                    nc.sync.dma_start(
                        out=s1_row[0:1, ts * P:(ts + 1) * P],
                        in_=s1c[:, ts:ts + 1])
                nc.gpsimd.partition_broadcast(xst["s1b"][:], s1_row[0:1, :])
                # cg = absm * mcl_g / 127 per token (column form)
                nc.vector.tensor_scalar(
                    out=xst["cg"][:], in0=xst["absm"][:],
                    scalar1=st["mclb"][:, 0:1],
                    scalar2=1.0 / 127.0, op0=A.mult, op1=A.mult)

            def emit_xquant_chunk(xst, ki_list):
                for ki in ki_list:
                    xt_sb = streamp.tile([P, T_CORE], F32, tag="xld")
                    nc.sync.dma_start(out=xt_sb[:], in_=xt_r[ki])
                    # v = x * s1 (Pool)
                    nc.gpsimd.tensor_tensor(
                        out=xt_sb[:], in0=xt_sb[:], in1=xst["s1b"][:], op=A.mult)
                    for hf in range(2):
                        sl = slice(hf * 512, (hf + 1) * 512)
                        # a = rne(v) bf16 (Pool)
                        a_sb = ewp.tile([P, 512], BF16, tag="atmp")
                        nc.gpsimd.tensor_scalar(
                            out=a_sb[:], in0=xt_sb[:, sl], scalar1=MROUND,
                            scalar2=-MROUND, op0=A.add, op1=A.add)
                        # u = M + rne(a/16) (ACT), hi16 = 16*rne(a/16) fp8
                        u_sb = ewp.tile([P, 512], F32, tag="sc512", bufs=3, name="u_sb")
                        nc.scalar.activation(
                            u_sb[:], a_sb[:], AF.Copy, scale=1.0 / 16.0,
                            bias=MROUND)
                        nc.scalar.activation(
                            xst["xhiT"][:, ki, sl], u_sb[:], AF.Copy,
                            scale=16.0, bias=-M16)
                        # lo = a - hi16 (DVE), exact, |lo| <= 8
                        nc.vector.tensor_tensor(
                            out=xst["xloT"][:, ki, sl], in0=a_sb[:],
                            in1=xst["xhiT"][:, ki, sl], op=A.subtract)

            # ---------------- main loop phases ----------------------------
            def gup_phase(tt, xst, wc):
                tsl = slice(tt * P, (tt + 1) * P)
                cg = xst["cg"][:, tt:tt + 1]
                prod = prodp.tile([P, NOB, OB], BF16, tag="prod")
                am8 = tinyp.tile([P, NOB], F32, tag="am8")
                for ob in range(NOB):
                    osl = slice(ob * OB, (ob + 1) * OB)
                    psg = pgp.tile([P, OB], F32, tag="pg")
                    psu = pgp.tile([P, OB], F32, tag="pg")
                    # DoubleRow streaks: full gate accumulation, then up.
                    for ps, wsb in ((psg, wc["wg"]), (psu, wc["wu"])):
                        for h, xh in enumerate((xst["xhiT"], xst["xloT"])):
                            for j in range(KI // 2):
                                nc.tensor.matmul(
                                    ps[:], lhsT=xh[:, 2 * j:2 * j + 2, tsl],
                                    rhs=wsb[:, 2 * j:2 * j + 2, osl],
                                    start=(h == 0 and j == 0),
                                    stop=(h == 1 and j == KI // 2 - 1),
                                    perf_mode=DR)
                    gsil = ewp.tile([P, OB], BF16, tag="gsil")
                    nc.scalar.activation(gsil[:], psg[:], AF.Silu, scale=cg)
                    nc.vector.tensor_tensor(
                        out=prod[:, ob], in0=gsil[:], in1=psu[:], op=A.mult)
                    nc.vector.tensor_reduce(
                        out=am8[:, ob:ob + 1], in_=prod[:, ob],
                        axis=mybir.AxisListType.X, op=A.max,
                        apply_absolute_value=True)

                amax = tinyp.tile([P, 1], F32, tag="amax")
                nc.vector.tensor_reduce(
                    out=amax[:], in_=am8[:], axis=mybir.AxisListType.X, op=A.max)
                nc.vector.tensor_scalar_max(amax[:], amax[:], 1e-5)
                s2col = tinyp.tile([P, 1], F32, tag="s2")
                nc.vector.reciprocal(out=s2col[:], in_=amax[:])
                nc.vector.tensor_scalar_mul(s2col[:], s2col[:], 127.0)
                fcol = tinyp.tile([P, 1], F32, tag="fcol")
                nc.vector.tensor_tensor(
                    out=fcol[:], in0=amax[:], in1=xst["absm"][:, tt:tt + 1],
                    op=A.mult)
                nc.vector.tensor_scalar_mul(fcol[:], fcol[:],
                                            xst["bc_ud"][:, 0:1])
                return prod, s2col, fcol

            def down_phase(tt, wc, prod, s2col, fcol):
                # quantize: two ScalarE passes (mult+round, unbias+cast)
                prodq = pqp.tile([P, NOB, OB], BF16, tag="prodq")
                for ob in range(NOB):
                    qtmp = ewp.tile([P, OB], F32, tag="sc512", bufs=3, name="qtmp")
                    nc.scalar.activation(
                        qtmp[:], prod[:, ob], AF.Copy,
                        scale=s2col[:, 0:1], bias=MROUND)
                    nc.scalar.activation(
                        prodq[:, ob], qtmp[:], AF.Copy, bias=-MROUND)

                # transpose prodq tiles back to [inner, token] for down;
                # evacuations alternate DVE/ACT to balance engine load
                pqT = pqp.tile([P, KO, P], BF16, tag="pqT")
                for j in range(KO):
                    pt_t = ptp.tile([P, P], BF16, tag="pt")
                    nc.tensor.transpose(
                        pt_t[:], prodq[:, j // 4, (j % 4) * P:(j % 4 + 1) * P],
                        ident[:])
                    if j % 2 == 0:
                        nc.vector.tensor_copy(out=pqT[:, j], in_=pt_t[:])
                    else:
                        nc.scalar.copy(out=pqT[:, j], in_=pt_t[:])

                # down projection: two bf16 streaks (no psum interleaving)
                psd0 = pdp.tile([P, 512], F32, tag="pd")
                psd1 = pdp.tile([P, 512], F32, tag="pd")
                for ko in range(KO):
                    nc.tensor.matmul(
                        psd0[:], lhsT=pqT[:, ko], rhs=wc["wd"][:, ko, 0:512],
                        start=(ko == 0), stop=(ko == KO - 1))
                for ko in range(KO):
                    nc.tensor.matmul(
                        psd1[:], lhsT=pqT[:, ko], rhs=wc["wd"][:, ko, 512:1024],
                        start=(ko == 0), stop=(ko == KO - 1))
                for hh, psd in ((0, psd0), (1, psd1)):
                    hsl = slice(hh * 512, (hh + 1) * 512)
                    osb = ewp.tile([P, 512], F32, tag="sc512", bufs=3, name="osb")
                    nc.scalar.activation(
                        osb[:], psd[:], AF.Copy, scale=fcol[:, 0:1])
                    nc.sync.dma_start(out=out_r[tt][:, hsl], in_=osb[:])

            # ---------------- rep orchestration ---------------------------
            st = stage_weights()
            xst = xst_new()
            xst["bc_ud"] = st["bc_ud"]
            emit_absmax_chunk(xst, range(NT))
            emit_s1(xst, st)
            emit_xquant_chunk(xst, range(KI))

            for rep in range(reps):
                more = rep + 1 < reps
                wc = emit_cache(st)
                nst = None
                nxst = None
                pending = None
                for tt in range(NT):
                    state = gup_phase(tt, xst, wc)
                    if more:
                        if tt == 0:
                            nst = stage_weights()
                        elif tt == 2:
                            nxst = xst_new()
                            nxst["bc_ud"] = nst["bc_ud"]
                            emit_absmax_chunk(nxst, range(0, 4))
                        elif tt == 3:
                            emit_absmax_chunk(nxst, range(4, NT))
                        elif tt == 4:
                            emit_s1(nxst, nst)
                            emit_xquant_chunk(nxst, range(0, 2))
                        elif tt in (5, 6, 7):
                            emit_xquant_chunk(nxst, range(2 * (tt - 4),
                                                          2 * (tt - 3)))
                    if pending is not None:
                        down_phase(tt - 1, wc, *pending)
                    pending = state
                down_phase(NT - 1, wc, *pending)
                if more:
                    st, xst = nst, nxst

    nc.compile()
    return nc


_NC_CACHE = {}


def _get_nc():
    if "nc" not in _NC_CACHE:
        _NC_CACHE["nc"] = build_bass(sim_mode=False)
    return _NC_CACHE["nc"]


def make_in_maps(x, w_gate, w_up, w_down):
    x2 = np.ascontiguousarray(
        np.asarray(x, dtype=np.float32).reshape(N_CORES * T_CORE, HID))
    xt = np.ascontiguousarray(x2.T)                       # [1024, 8192]
    wgt = np.ascontiguousarray(np.asarray(w_gate, dtype=np.float32).T)
    wut = np.ascontiguousarray(np.asarray(w_up, dtype=np.float32).T)
    wdt = np.ascontiguousarray(np.asarray(w_down, dtype=np.float32).T)
    in_maps = []
    for c in range(N_CORES):
        in_maps.append({
            "x_shard": np.ascontiguousarray(x2[c * T_CORE:(c + 1) * T_CORE]),
            "xT_shard": np.ascontiguousarray(
                xt[:, c * T_CORE:(c + 1) * T_CORE]),
            "wgT_shard": np.ascontiguousarray(
                wgt[:, c * OSH:(c + 1) * OSH]),
            "wuT_shard": np.ascontiguousarray(
                wut[:, c * OSH:(c + 1) * OSH]),
            "wdT_shard": np.ascontiguousarray(
                wdt[c * WDSH:(c + 1) * WDSH, :]),
        })
    return in_maps


def assemble_output(results):
    parts = [results[c]["out_shard"] for c in range(N_CORES)]
    return np.concatenate(parts, axis=0).reshape(4, 2048, HID)


def kernel(x, w_gate, w_up, w_down):
    from concourse.bass_utils import run_bass_kernel_spmd
    nc = _get_nc()
    in_maps = make_in_maps(x, w_gate, w_up, w_down)
    res = run_bass_kernel_spmd(nc, in_maps, list(range(N_CORES)), trace=False)
    return assemble_output(res.results)
